# revision 4
# baseline (speedup 1.0000x reference)
"""2-layer GCN with residual (GCNResnet) on 8 Trainium2 NeuronCores.

Strategy (graph/data parallel, per the sharding hint):
- Nodes are padded to N_pad = 8*BPC*128 and sharded contiguously: core c owns
  BPC blocks of 128 destination nodes; edges are assigned to the core owning
  their dst. The small weight matrices are replicated.
- deg / 1/sqrt(deg) and all edge bucketing is host-side preprocessing of the
  graph structure (independent of feature data).
- symmetric norm factorizes: norm_e = dinv[src]*dinv[dst]. The src factor is
  folded into the gather tables (h1, h2 rows pre-scaled by dinv), the dst
  factor into the selection matrix S.
- Layer 1: every core redundantly computes h1 = (x@W1)*dinv for ALL nodes
  (cheap) into a row-major DRAM table; per dst-block, rows h1[src_e] are
  fetched with dma_gather and reduced with PE matmuls:
      aggT[hid, loc] += G_e[e, hid]^T @ S_e[e, loc],
  S_e[e, loc] = (dst_local[e]==loc) * dinv[dst_e] built in one DVE op from an
  iota matrix. Bias + leaky-relu applied per block, then
  h2 = (lrelu(aggT+b1))^T @ W2 per block, scaled by dinv, written to the
  core's h2 shard.
- One AllGather assembles the full h2 table; layer 2 repeats gather+matmul
  aggregation (out[loc, feat] += S_e^T @ G2_e) and adds the (x + b2) residual.
- dma_gather indices are int16, so gather tables are addressed in two halves
  (rows < HALF and >= HALF) with separate gathers.
"""

import sys

sys.path.insert(0, "/opt/trn_rl_repo")

import numpy as np
import ml_dtypes

import concourse.bacc as bacc
import concourse.bass as bass
import concourse.mybir as mybir
import concourse.tile as tile
from concourse.bass_utils import run_bass_kernel_spmd

P = 128
NCORES = 8

BF16 = mybir.dt.bfloat16
F32 = mybir.dt.float32
I16 = mybir.dt.int16

NEG_SLOPE = 0.01


# ----------------------------------------------------------------- host prep
def _preprocess(edge_index, n_nodes, grp):
    """Bucket edges by (dst block, src half), pad to 128-multiples with sizes
    shared across cores, and build all per-core index/metadata arrays."""
    n = n_nodes
    bpc = -(-n // (NCORES * P))  # blocks per core
    nblk = NCORES * bpc
    n_pad = nblk * P
    half = (nblk // 2) * P
    assert half <= 32768 and (n_pad - half) <= 32768

    src = np.concatenate([edge_index[0], np.arange(n, dtype=np.int64)]).astype(np.int64)
    dst = np.concatenate([edge_index[1], np.arange(n, dtype=np.int64)]).astype(np.int64)
    e_all = src.shape[0]

    # dst already includes self loops -> deg >= 1 (matches ref's segment_sum)
    deg = np.bincount(dst, minlength=n).astype(np.float32)
    dinv = np.where(deg > 0, 1.0 / np.sqrt(np.maximum(deg, 1e-12)), 0.0).astype(
        np.float32
    )

    blk = (dst // P).astype(np.int64)
    hlf = (src >= half).astype(np.int64)
    key = blk * 2 + hlf  # 2*nblk buckets
    counts = np.bincount(key, minlength=2 * nblk)
    cnt = counts.reshape(nblk, 2)  # [block, half]
    cnt_cs = cnt.reshape(NCORES, bpc, 2)

    # shared-across-cores tile counts per (slot, half)
    tlo = np.ceil(cnt_cs[:, :, 0].max(axis=0) / P).astype(np.int64)  # [bpc]
    thi = np.ceil(cnt_cs[:, :, 1].max(axis=0) / P).astype(np.int64)

    # groups of consecutive slots share one gather per half
    groups = []
    s = 0
    while s < bpc:
        e = min(s + grp, bpc)
        groups.append(list(range(s, e)))
        s = e

    # e-tile layout: for each group: [lo tiles of slots][hi tiles of slots]
    slot_lo_t0 = np.zeros(bpc, np.int64)  # first e-tile of slot's lo run
    slot_hi_t0 = np.zeros(bpc, np.int64)
    gathers = []  # (half, tile0, ntiles, slots)
    t = 0
    for g in groups:
        g_lo0 = t
        for s_ in g:
            slot_lo_t0[s_] = t
            t += tlo[s_]
        if t > g_lo0:
            gathers.append((0, g_lo0, t - g_lo0, list(g)))
        g_hi0 = t
        for s_ in g:
            slot_hi_t0[s_] = t
            t += thi[s_]
        if t > g_hi0:
            gathers.append((1, g_hi0, t - g_hi0, list(g)))
    tt = int(t)  # total e-tiles per core
    tote = tt * P

    # base edge offset for each (slot, half) — identical for every core
    base = np.zeros((bpc, 2), np.int64)
    for s_ in range(bpc):
        base[s_, 0] = slot_lo_t0[s_] * P
        base[s_, 1] = slot_hi_t0[s_] * P

    # per-edge padded position inside its core's edge array
    order = np.argsort(key, kind="stable")
    sk = key[order]
    seg_start = np.concatenate([[0], np.cumsum(counts)[:-1]])
    rank_sorted = np.arange(e_all, dtype=np.int64) - seg_start[sk]
    rank = np.empty(e_all, np.int64)
    rank[order] = rank_sorted

    core = blk // bpc
    slot = blk % bpc
    padpos = base[slot, hlf] + rank

    idx_arr = np.zeros((NCORES, tote), np.int16)
    dl_arr = np.full((NCORES, tote), -1.0, np.float32)
    dv_arr = np.zeros((NCORES, tote), np.float32)
    idx_arr[core, padpos] = (src - hlf * half).astype(np.int16)
    dl_arr[core, padpos] = (dst % P).astype(np.float32)
    dv_arr[core, padpos] = dinv[dst]

    # wrapped int16 index layout: idx j of a gather -> [j%16 (+16r), col0+j//16]
    idx16 = np.zeros((NCORES, P, tote // 16), np.int16)
    for hf, t0, nt, _slots in gathers:
        e0, e1 = t0 * P, (t0 + nt) * P
        seg = idx_arr[:, e0:e1].reshape(NCORES, (e1 - e0) // 16, 16)
        wrapped = seg.transpose(0, 2, 1)  # [NC, 16, n/16]
        idx16[:, :, e0 // 16 : e1 // 16] = np.tile(wrapped, (1, 8, 1))

    dstloc = dl_arr.reshape(NCORES, tt, P).transpose(0, 2, 1).copy()  # [NC,128,tt]
    dinvd = dv_arr.reshape(NCORES, tt, P).transpose(0, 2, 1).copy()

    # per-slot consumption ranges
    slots = [
        dict(lo0=int(slot_lo_t0[s_]), nlo=int(tlo[s_]), hi0=int(slot_hi_t0[s_]),
             nhi=int(thi[s_]))
        for s_ in range(bpc)
    ]

    dinv_pad = np.zeros(n_pad, np.float32)
    dinv_pad[:n] = dinv
    dinv_t = dinv_pad.reshape(nblk, P).T.copy()  # [128, nblk]
    dinv_c = dinv_pad.reshape(NCORES, bpc, P).transpose(0, 2, 1).copy()  # [NC,128,bpc]

    plan = dict(
        n=n, n_pad=n_pad, bpc=bpc, nblk=nblk, half=half, tt=tt,
        gathers=gathers, slots=slots,
    )
    percore = dict(idx16=idx16, dstloc=dstloc, dinvd=dinvd, dinv_c=dinv_c)
    return plan, percore, dinv_t


# ------------------------------------------------------------ program build
def _build_program(plan, feat, hid):
    n_pad, bpc, nblk, half, tt = (
        plan["n_pad"], plan["bpc"], plan["nblk"], plan["half"], plan["tt"]
    )
    nsh = bpc * P  # rows per core shard
    assert feat <= P and hid == P

    nc = bacc.Bacc("TRN2", target_bir_lowering=False, debug=False,
                   num_devices=NCORES)

    # inputs (replicated)
    x_T = nc.dram_tensor("x_T", [feat, n_pad], BF16, kind="ExternalInput")
    W1 = nc.dram_tensor("W1", [feat, hid], BF16, kind="ExternalInput")
    b1 = nc.dram_tensor("b1", [P, 1], F32, kind="ExternalInput")
    W2p = nc.dram_tensor("W2p", [hid, P], BF16, kind="ExternalInput")
    dinv_t = nc.dram_tensor("dinv_t", [P, nblk], F32, kind="ExternalInput")
    iota = nc.dram_tensor("iota", [P, P], F32, kind="ExternalInput")
    # inputs (per core)
    idx16 = nc.dram_tensor("idx16", [P, tt * P // 16], I16, kind="ExternalInput")
    dstloc = nc.dram_tensor("dstloc", [P, tt], F32, kind="ExternalInput")
    dinvd = nc.dram_tensor("dinvd", [P, tt], F32, kind="ExternalInput")
    dinv_c = nc.dram_tensor("dinv_c", [P, bpc], F32, kind="ExternalInput")
    xb2 = nc.dram_tensor("xb2", [nsh, feat], F32, kind="ExternalInput")
    # output
    out = nc.dram_tensor("out", [nsh, feat], F32, kind="ExternalOutput")

    # internal DRAM
    h1 = nc.dram_tensor("h1", [n_pad, hid], BF16)
    h2sh = nc.dram_tensor("h2sh", [nsh, P], BF16)
    h2full = nc.dram_tensor("h2full", [n_pad, P], BF16, addr_space="Shared")

    h1r = h1.rearrange("(n p) d -> p n d", p=P)  # [128, nblk, hid]
    outr = out.rearrange("(n p) d -> p n d", p=P)  # [128, bpc, feat]
    h2shr = h2sh.rearrange("(n p) d -> p n d", p=P)  # [128, bpc, 128]

    gathers = plan["gathers"]
    slots = plan["slots"]
    max_g = max(g[2] for g in gathers)

    with tile.TileContext(nc) as tc:
        with (
            tc.tile_pool(name="const", bufs=1) as cpool,
            tc.tile_pool(name="xt", bufs=2) as xtpool,
            tc.tile_pool(name="stage", bufs=2) as stpool,
            tc.tile_pool(name="gt", bufs=3) as gpool,
            tc.tile_pool(name="s", bufs=6) as spool,
            tc.tile_pool(name="work", bufs=3) as wpool,
            tc.tile_pool(name="psA", bufs=2, space="PSUM") as psA,
            tc.tile_pool(name="psB", bufs=2, space="PSUM") as psB,
            tc.tile_pool(name="psC", bufs=2, space="PSUM") as psC,
        ):
            # ---- resident constants / metadata
            W1_sb = cpool.tile([feat, hid], BF16)
            nc.sync.dma_start(out=W1_sb[:], in_=W1[:])
            W2_sb = cpool.tile([hid, P], BF16)
            nc.sync.dma_start(out=W2_sb[:], in_=W2p[:])
            b1_sb = cpool.tile([P, 1], F32)
            nc.sync.dma_start(out=b1_sb[:], in_=b1[:])
            iota_sb = cpool.tile([P, P], F32)
            nc.sync.dma_start(out=iota_sb[:], in_=iota[:])
            dinvt_sb = cpool.tile([P, nblk], F32)
            nc.sync.dma_start(out=dinvt_sb[:], in_=dinv_t[:])
            dinvc_sb = cpool.tile([P, bpc], F32)
            nc.sync.dma_start(out=dinvc_sb[:], in_=dinv_c[:])
            idx_sb = cpool.tile([P, tt * P // 16], I16)
            nc.sync.dma_start(out=idx_sb[:], in_=idx16[:])
            dl_sb = cpool.tile([P, tt], F32)
            nc.sync.dma_start(out=dl_sb[:], in_=dstloc[:])
            dv_sb = cpool.tile([P, tt], F32)
            nc.sync.dma_start(out=dv_sb[:], in_=dinvd[:])
            xb2_sb = cpool.tile([P, bpc, feat], F32)
            nc.sync.dma_start(
                out=xb2_sb[:], in_=xb2.rearrange("(n p) d -> p n d", p=P)[:]
            )

            # ---- phase 0: h1 = (x @ W1) * dinv  for ALL node blocks
            XCH = 32  # node blocks per x_T chunk
            for c0 in range(0, nblk, XCH):
                c1 = min(c0 + XCH, nblk)
                xt = xtpool.tile([feat, XCH * P], BF16, tag="xt")
                nc.sync.dma_start(out=xt[:, : (c1 - c0) * P],
                                  in_=x_T[:, c0 * P : c1 * P])
                for j0 in range(c0, c1, 8):
                    j1 = min(j0 + 8, c1)
                    st = stpool.tile([P, 8, hid], BF16, tag="h1st")
                    for j in range(j0, j1):
                        ps = psA.tile([P, hid], F32, space="PSUM", tag="ps_ph0")
                        nc.tensor.matmul(
                            out=ps[:],
                            lhsT=xt[:, (j - c0) * P : (j - c0 + 1) * P],
                            rhs=W1_sb[:],
                            start=True, stop=True,
                        )
                        nc.vector.tensor_scalar(
                            out=st[:, j - j0, :], in0=ps[:],
                            scalar1=dinvt_sb[:, j : j + 1], scalar2=None,
                            op0=mybir.AluOpType.mult,
                        )
                    nc.sync.dma_start(out=h1r[:, j0:j1, :],
                                      in_=st[:, : j1 - j0, :])

            # ---- layer 1 + h2 per own block; layer 2 after allgather
            def aggregate(layer, table, elem, g_dt, out_cb):
                """gather+matmul aggregation over this core's blocks.
                out_cb(b, psum_ap) consumes the [.,.] PSUM of block b."""
                # issue gathers per (group, half)
                gt = {}
                for hf, t0, ntl, _slots_g in gathers:
                    g = gpool.tile([P, max_g, elem], g_dt, tag=f"g{layer}", name=f"g{layer}_{hf}_{t0}")
                    src_ap = table[hf * half : hf * half + half, :]
                    nc.gpsimd.dma_gather(
                        out_ap=g[:, :ntl, :],
                        in_ap=src_ap,
                        idxs_ap=idx_sb[:, t0 * 8 : (t0 + ntl) * 8],
                        num_idxs=ntl * P,
                        num_idxs_reg=ntl * P,
                        elem_size=elem,
                        single_packet=False,
                    )
                    gt[(hf, t0)] = g

                for b, sl in enumerate(slots):
                    runs = []
                    if sl["nlo"]:
                        runs.append((0, sl["lo0"], sl["nlo"]))
                    if sl["nhi"]:
                        runs.append((1, sl["hi0"], sl["nhi"]))
                    # locate the gather tile each run lives in
                    nt_b = sum(r[2] for r in runs)
                    assert nt_b > 0
                    if layer == 1:
                        ps = psB.tile([P, P], F32, space="PSUM", tag="ps_agg")
                    else:
                        ps = psA.tile([P, feat], F32, space="PSUM", tag="ps_ph0")
                    k = 0
                    for hf, t0, ntl in runs:
                        # find owning gather
                        own = None
                        for hf2, gt0, gnt, _s in gathers:
                            if hf2 == hf and gt0 <= t0 and t0 + ntl <= gt0 + gnt:
                                own = (hf2, gt0)
                                break
                        g = gt[own]
                        goff = t0 - own[1]
                        for i in range(ntl):
                            tcol = t0 + i
                            s_t = spool.tile([P, P], g_dt, tag=f"s{layer}")
                            nc.vector.tensor_scalar(
                                out=s_t[:], in0=iota_sb[:],
                                scalar1=dl_sb[:, tcol : tcol + 1],
                                scalar2=dv_sb[:, tcol : tcol + 1],
                                op0=mybir.AluOpType.is_equal,
                                op1=mybir.AluOpType.mult,
                            )
                            if layer == 1:
                                nc.tensor.matmul(
                                    out=ps[:], lhsT=g[:, goff + i, :], rhs=s_t[:],
                                    start=(k == 0), stop=(k == nt_b - 1),
                                )
                            else:
                                nc.tensor.matmul(
                                    out=ps[:], lhsT=s_t[:],
                                    rhs=g[:, goff + i, :feat],
                                    start=(k == 0), stop=(k == nt_b - 1),
                                )
                            k += 1
                    out_cb(b, ps)

            # layer 1 consumers: lrelu + h2 + scale, staged h2 writes
            h2_stage = {}

            def l1_out(b, ps):
                z = wpool.tile([P, P], F32, tag="z")
                nc.vector.tensor_scalar(
                    out=z[:], in0=ps[:], scalar1=b1_sb[:, :1], scalar2=None,
                    op0=mybir.AluOpType.add,
                )
                a1 = wpool.tile([P, P], BF16, tag="a1")
                nc.vector.scalar_tensor_tensor(
                    out=a1[:], in0=z[:], scalar=NEG_SLOPE, in1=z[:],
                    op0=mybir.AluOpType.mult, op1=mybir.AluOpType.max,
                )
                ps2 = psC.tile([P, P], F32, space="PSUM", tag="ps_h2")
                nc.tensor.matmul(out=ps2[:], lhsT=a1[:], rhs=W2_sb[:],
                                 start=True, stop=True)
                j0 = (b // 8) * 8
                if j0 not in h2_stage:
                    h2_stage[j0] = stpool.tile([P, 8, P], BF16, tag="h2st", name=f"h2st{j0}")
                st = h2_stage[j0]
                nc.vector.tensor_scalar(
                    out=st[:, b - j0, :], in0=ps2[:],
                    scalar1=dinvc_sb[:, b : b + 1], scalar2=None,
                    op0=mybir.AluOpType.mult,
                )
                if b == min(j0 + 7, bpc - 1):
                    nc.sync.dma_start(out=h2shr[:, j0 : b + 1, :],
                                      in_=st[:, : b + 1 - j0, :])

            aggregate(1, h1, hid, BF16, l1_out)

            # allgather h2 shards -> full table
            nc.gpsimd.collective_compute(
                "AllGather",
                mybir.AluOpType.bypass,
                replica_groups=[list(range(NCORES))],
                ins=[h2sh[:]],
                outs=[h2full[:]],
            )

            out_stage = {}

            def l2_out(b, ps):
                j0 = (b // 8) * 8
                if j0 not in out_stage:
                    out_stage[j0] = stpool.tile([P, 8, feat], F32, tag="outst", name=f"outst{j0}")
                st = out_stage[j0]
                nc.vector.tensor_tensor(
                    out=st[:, b - j0, :], in0=ps[:], in1=xb2_sb[:, b, :],
                    op=mybir.AluOpType.add,
                )
                if b == min(j0 + 7, bpc - 1):
                    nc.sync.dma_start(out=outr[:, j0 : b + 1, :],
                                      in_=st[:, : b + 1 - j0, :])

            aggregate(2, h2full, P, BF16, l2_out)

    nc.compile()
    return nc


# ------------------------------------------------------------------- driver
_CACHE = {}


def _get_compiled(edge_index, n, feat, hid, grp=4):
    key = (hash(edge_index.tobytes()), n, feat, hid, grp)
    if key not in _CACHE:
        plan, percore, dinv_t = _preprocess(edge_index, n, grp)
        nc = _build_program(plan, feat, hid)
        _CACHE[key] = (plan, percore, dinv_t, nc)
    return _CACHE[key]


def kernel(x, W1, b1, W2, b2, edge_index):
    x = np.asarray(x, np.float32)
    W1 = np.asarray(W1, np.float32)
    b1 = np.asarray(b1, np.float32)
    W2 = np.asarray(W2, np.float32)
    b2 = np.asarray(b2, np.float32)
    edge_index = np.asarray(edge_index)

    n, feat = x.shape
    hid = W1.shape[1]
    plan, percore, dinv_t, nc = _get_compiled(edge_index, n, feat, hid)
    n_pad, bpc = plan["n_pad"], plan["bpc"]
    nsh = bpc * P

    x_T = np.zeros((feat, n_pad), ml_dtypes.bfloat16)
    x_T[:, :n] = x.T.astype(ml_dtypes.bfloat16)
    W1b = W1.astype(ml_dtypes.bfloat16)
    W2p = np.zeros((hid, P), ml_dtypes.bfloat16)
    W2p[:, :feat] = W2.astype(ml_dtypes.bfloat16)
    b1c = b1.reshape(hid, 1).astype(np.float32)
    iota = np.broadcast_to(np.arange(P, dtype=np.float32), (P, P)).copy()
    xb2 = np.zeros((n_pad, feat), np.float32)
    xb2[:n] = x + b2[None, :]

    in_maps = []
    for c in range(NCORES):
        in_maps.append(dict(
            x_T=x_T, W1=W1b, b1=b1c, W2p=W2p, dinv_t=dinv_t, iota=iota,
            idx16=percore["idx16"][c], dstloc=percore["dstloc"][c],
            dinvd=percore["dinvd"][c], dinv_c=percore["dinv_c"][c],
            xb2=xb2[c * nsh : (c + 1) * nsh],
        ))

    res = run_bass_kernel_spmd(nc, in_maps, list(range(NCORES)))
    out = np.concatenate([res.results[c]["out"] for c in range(NCORES)], axis=0)
    return out[:n].astype(np.float32)
